# revision 15
# baseline (speedup 1.0000x reference)
"""MoE MLP (top-2 routing, capacity 1.25) on 8 Trainium2 NeuronCores.

Strategy (expert-parallel, per the sharding hint):
  - Router + top-k + capacity assignment run on host in float64 (cheap:
    0.27 GFLOP vs 344 GFLOP for the expert FFNs, and data-dependent
    control flow is a poor fit for the static Bass dataflow graph).
  - Every expert overflows capacity for this problem size (mean load
    4096 assignments vs cap 2560), so each of the 8 cores computes a
    dense [cap,D] @ [D,F] -> gelu -> [cap,F] @ [F,D] FFN for one expert.
  - Dispatch/combine (gather/scatter by routing indices) run on host.

Device kernel layout: activations are kept feature-major ([D, cap] /
[F, cap]) so both matmuls use weight tiles as the stationary operand and
no transposes are needed anywhere.  Two variants (MODE below):
  - "bf16" (default): weights cast to bf16 on host and kept resident in
    SBUF; minimal DMA, pure PE-bound.  2560 N=512 matmuls ~= 553us floor
    at 78.6 TF/s; measured ~570us.  Startup is the optimization
    battleground: the NEFF preamble + HW-DGE kick keep data from flowing
    before ~8.2us and the ring bandwidth ramps slowly, so t=0's mm1 runs
    as split-K waves whose operands are issued as the first ring
    descriptors in exact consumption order (see _build_nc), a junk-MM
    burst holds the HAM clock-gate at 2.4 GHz through the ramp, and the
    bias loads ride the main ring as single linear descriptors (SWDGE
    scatter-packets were stealing early bandwidth).  Output returns as
    bf16 to halve the out-DMA.  rel err ~3.8e-3.
  - "f32r": float32r matmuls, weights streamed from HBM every token
    tile; ~607us, rel err ~2.1e-4.  Kept as the high-precision fallback.

Optimization notes (second session), for whoever picks this up next:
  - Measured per-MM issue spacing is N/f_eff + ~2.7ns; f_eff varies with
    device power state between ~2.2 and 2.4 GHz run-to-run (SW throttle,
    not HAM).  The same NEFF measured 569.6us and 623.5us hours apart.
    Always re-baseline before comparing variants.
  - fp8 e4m3 DoubleRow measures exactly 2x bf16 FLOPs/inst (K=256 per
    pass at unchanged spacing), NOT the cost model's 0.5 cyc/row (4x).
    At 2x, residual-corrected fp8 breaks even at best, and plain fp8
    fails the gate: each uncorrected e4m3 operand stream contributes
    ~2.6% rel err (measured: mm1-only 3.9e-2, mm2-only 3.8e-2, both
    5.4e-2 vs gate 2e-2).  Dead end unless the gate loosens >3x.
  - MODE="strassen" below implements 1-level <2,2,2> on both matmuls
    (2240 slot-equivalents vs 2560): numerics exactly as simulated
    (7.1e-3), PE runs at full 2.4GHz, but matmuls whose operands come
    from freshly-written/streamed SBUF tiles (mm2's h/V rhs, streamed
    weight units) lose the LDWEIGHTS pull-ahead and serialize at
    +95ns/MM; plus the w2-form stream wants ~290GB/s in-phase.  Measured
    741us (sequential phases) / 774us (mm2(t) interleaved with
    mm1(t+1)).  Ideas not yet tried: single-instruction V-builds via 3D
    APs (one writer -> one wait instead of 16), coarser h dependencies,
    SBUF placement control to separate LDW and rhs read ports.
  - PSUM facts validated on hw: two accumulation groups co-packed per
    bank work iff strictly sequential (lazy bank-granular pending-zero);
    DVE tensor_tensor accepts at most ONE PSUM operand (NCC_IBVF027).
"""

import numpy as np
import ml_dtypes

B, T, D, F_FF, E, TOP_K = 8, 2048, 1024, 4096, 8, 2
N = B * T
CAP = 2560          # int(1.25 * N / E)
NCORES = 8
P = 128
DC = D // P         # 8 chunks of the model dim
FC = F_FF // P      # 32 chunks of the ff dim
NT = 512            # token tile (one PSUM bank of fp32)
TT = CAP // NT      # 5 token tiles
FB = 512            # f-column block for w1 loads
NFB = F_FF // FB

BF16 = ml_dtypes.bfloat16

# "bf16" (default): dense bf16, weights resident in SBUF
# "strassen": 1-level Strassen on both FFN matmuls -- 7/8 the PE slots and
#   numerically fine (7.1e-3), but measured SLOWER on hw (741us vs 622us):
#   matmuls whose weights come from freshly-streamed SBUF tiles lose the
#   LDWEIGHTS pull-ahead (~+95ns on most mm2 MMs, pattern consistent with
#   SBUF read-port contention between the LDW stream and the rhs stream),
#   and the w2-form stream needs ~290GB/s against ~250 deliverable.  Kept
#   for reference / future work.
# "f32r": weights streamed, float32r matmuls (higher precision fallback)
MODE = "bf16"

# Strassen <2,2,2> geometry (per 512-token tile: tok-halves of 256)
HT = 256            # tok half-tile = product moving width
K1 = 4              # mm1 k-tiles per product (D-half 512 = 4x128)
JC = 16             # mm1 f-chunks (F-half 2048 = 16x128)
K2 = 16             # mm2 k-tiles per product (F-half 2048)
DJ = 4              # mm2 d-chunks (D-half 512 = 4x128)
# mm2 product order: plain forms (H11, H22) first so the DVE has PE-time to
# build the combined V forms before their products start
SEQ2 = [2, 3, 0, 1, 4, 5, 6]
JUNK_MM = 12        # warm-up matmuls: unbroken burst from ~7.8us that
                    # covers a full HAM busy window with margin (PE at
                    # 2.4 GHz before the split-K waves start); with the
                    # bias loads off SWDGE, wave delivery never starves
                    # after that.  A/B: 8 -> 570.6, 12 -> 569.9 +-0.5
                    # (n=6), 10 -> 569.7 (n=1, within the 12 band)

_NC_CACHE = {}


def _build_nc():
    """Per-core Bass graph: dense FFN for one expert (SPMD across 8 cores).

    bf16 weights resident in SBUF; host pre-packs x/w1/w2 so every DMA
    descriptor is a fully contiguous HBM read.  The PE queue is in-order,
    so a full-contraction first group would block until the whole t=0
    x tile + w1 block 0 (2.1 MB) lands (~18us at the ~220 GB/s the DMA
    ramps through early on).  Instead t=0's mm1 runs as 4 c-major waves
    of 8 fc-groups across all eight PSUM banks, with the DMA descriptors
    issued in exactly that consumption order: every 256KB w1-pair
    arrival unlocks 8 matmuls, the PE tracks the DMA ramp with no famine
    window, and a 12-matmul warm-up burst (started off a GPSIMD memset,
    the earliest engine out of the preamble) completes a full HAM busy
    window so everything runs at 2.4 GHz."""
    from contextlib import ExitStack

    import concourse.mybir as mybir
    import concourse.tile as tile
    from concourse import bacc

    bf = mybir.dt.bfloat16
    f32 = mybir.dt.float32
    AF = mybir.ActivationFunctionType

    nc = bacc.Bacc(trn_type="TRN2")
    # pre-packed: xT[t,c,p,n], w1[fb,cpair,p,2,f], w2[cg,p,4,d] -- the
    # leading axes index DMA descriptors, each one linear in HBM
    xT = nc.dram_tensor("xT", [TT, DC, P, NT], bf, kind="ExternalInput").ap()
    w1 = nc.dram_tensor("w1", [NFB, DC // 2, P, 2, FB], bf,
                        kind="ExternalInput").ap()
    w2 = nc.dram_tensor("w2", [FC // 4, P, 4, D], bf,
                        kind="ExternalInput").ap()
    # biases pre-packed to their SBUF layout so each loads as ONE linear
    # HWDGE descriptor (SWDGE scattered them as 112x 256B packets right
    # through the critical startup window, stealing early DMA bandwidth)
    b1 = nc.dram_tensor("b1", [P, FC], f32, kind="ExternalInput").ap()
    b2 = nc.dram_tensor("b2", [P, DC], f32, kind="ExternalInput").ap()
    out = nc.dram_tensor("out", [D, CAP], bf, kind="ExternalOutput").ap()

    with tile.TileContext(nc) as tc, ExitStack() as ctx:
        wpool = ctx.enter_context(tc.tile_pool(name="weights", bufs=1))
        xpool = ctx.enter_context(tc.tile_pool(name="xin", bufs=2))
        hpool = ctx.enter_context(tc.tile_pool(name="hmid", bufs=1))
        ypool = ctx.enter_context(tc.tile_pool(name="yout", bufs=4))
        # ph + py are distinct tags; 4 bufs each = all 8 PSUM banks
        ppool = ctx.enter_context(tc.tile_pool(name="psum", bufs=4, space="PSUM"))

        # PE warm-up first: memset on GPSIMD (earliest preamble exit, vs
        # ~2.4us later via the vector engine) so the junk-matmul burst
        # starts ~6us in, completes a full HAM busy window (PE at
        # 2.4 GHz), and ends right around first-DMA arrival.
        warm = wpool.tile([P, NT], bf)
        nc.gpsimd.memset(warm, 0.0)
        # 8 open prologue accumulators: 4 on the ph banks + 4 on the py
        # banks (idle during mm1 anyway)
        ph_pro = [ppool.tile([P, NT], f32, name=f"ph_pro{i}",
                             tag=("ph" if i < 4 else "py"))
                  for i in range(8)]
        for _ in range(JUNK_MM):
            nc.tensor.matmul(ph_pro[0], lhsT=warm[:, :P], rhs=warm,
                             start=True, stop=True)

        w1_s = wpool.tile([P, DC, F_FF], bf)

        def load_w1(fb, j):
            # single SP-ring: a second HWDGE ring (nc.scalar) was tried
            # and lost ~1us -- the 16 DMA engines are shared, so a second
            # ring steals exactly the bandwidth it adds, plus overhead
            nc.sync.dma_start(
                out=w1_s[:, 2 * j:2 * j + 2, fb * FB:(fb + 1) * FB],
                in_=w1[fb, j],
            )

        def load_x(t, c):
            nc.sync.dma_start(out=x_tiles[t][:, c, :], in_=xT[t, c])

        x_tiles = {0: xpool.tile([P, DC, NT], bf, name="x_s0", tag="xs")}

        # ---- DMA order matched to the t=0 wave consumption order: x and
        # the first two w1 blocks interleaved per-c (wave 0), then the
        # later blocks pair-by-pair in wave order
        b1_s = wpool.tile([P, FC], f32)
        b2_s = wpool.tile([P, DC], f32)
        for c in range(DC):
            load_x(0, c)
            if c % 2 == 0:
                load_w1(0, c // 2)
                load_w1(1, c // 2)
            if c == 3:
                # 20KB of biases mid-prefix: lands well before the first
                # Gelu (~17us) while delaying the prefix tail by <100ns
                nc.sync.dma_start(out=b1_s, in_=b1)
                nc.sync.dma_start(out=b2_s, in_=b2)
        for w in range(1, 4):
            for j in range(DC // 2):
                load_w1(2 * w, j)
                load_w1(2 * w + 1, j)

        # w2 resident, 8 linear chunks (needed from the first mm2 phase
        # ~70us in; lands well before that behind the prefix + w1)
        w2_s = wpool.tile([P, FC, D], bf)
        for cg in range(FC // 4):
            nc.sync.dma_start(out=w2_s[:, cg * 4:(cg + 1) * 4, :],
                              in_=w2[cg])

        # PE warm-up: junk matmuls bridging memset -> first-DMA arrival so
        # the HAM clock-gate starts its busy window early; the staggered
        # real matmuls keep it busy from there (no >3.4us idle anywhere).
        for t in range(TT):
            x_s = x_tiles[t]
            if t + 1 < TT:
                x_tiles[t + 1] = xpool.tile([P, DC, NT], bf,
                                            name=f"x_s{t + 1}", tag="xs")
                for c in range(DC):
                    load_x(t + 1, c)
            # h.T tile [f, tok] for this token tile
            h_s = hpool.tile([P, FC, NT], bf)

            def mm1_act(fc, ph):
                nc.scalar.activation(
                    h_s[:, fc, :], ph, AF.Gelu, bias=b1_s[:, fc:fc + 1]
                )

            fc0 = 0
            if t == 0:
                # t=0 mm1 runs as 4 c-major waves of 8 fc-groups across
                # all eight PSUM banks: every 256KB w1-pair arrival
                # unlocks 8 matmuls, so the PE tracks the DMA ramp with
                # no famine window (and so no HAM re-throttle), with the
                # maximum amount of work pulled ahead of each descriptor
                for wv in range(4):
                    ph_w = (ph_pro if wv == 0 else
                            [ppool.tile([P, NT], f32, name=f"ph_w{wv}_{g}",
                                        tag=("ph" if g < 4 else "py"))
                             for g in range(8)])
                    for c in range(DC):
                        for g in range(8):
                            fc = wv * 8 + g
                            nc.tensor.matmul(
                                ph_w[g],
                                lhsT=w1_s[:, c, fc * P:(fc + 1) * P],
                                rhs=x_s[:, c, :],
                                start=(c == 0),
                                stop=(c == DC - 1),
                            )
                    for g in range(8):
                        mm1_act(wv * 8 + g, ph_w[g])
                fc0 = FC
            for fc in range(fc0, FC):
                ph = ppool.tile([P, NT], f32, tag="ph")
                for c in range(DC):
                    nc.tensor.matmul(
                        ph,
                        lhsT=w1_s[:, c, fc * P:(fc + 1) * P],
                        rhs=x_s[:, c, :],
                        start=(c == 0),
                        stop=(c == DC - 1),
                    )
                mm1_act(fc, ph)
            for dc in range(DC):
                # split the kernel's final group so less serial ACT+DMA
                # trails the last matmul
                halves = 2 if (t == TT - 1 and dc == DC - 1) else 1
                w = NT // halves
                for s in range(halves):
                    # mm2 rotates through all 8 PSUM banks (ph banks are
                    # idle during this phase) so the bank-free wait never
                    # trails the ACT at group boundaries
                    py = ppool.tile([P, NT], f32, name="py",
                                    tag=("py" if dc % 2 else "ph"))
                    for fc in range(FC):
                        nc.tensor.matmul(
                            py[:, :w],
                            lhsT=w2_s[:, fc, dc * P:(dc + 1) * P],
                            rhs=h_s[:, fc, s * w:(s + 1) * w],
                            start=(fc == 0),
                            stop=(fc == FC - 1),
                        )
                    y_s = ypool.tile([P, NT], bf, name="y_s", tag="ys")
                    nc.scalar.activation(
                        y_s[:, :w], py[:, :w], AF.Identity,
                        bias=b2_s[:, dc:dc + 1]
                    )
                    # single final descriptor: splitting it (16KB primer +
                    # remainder) was tried and lost 1.2us -- the second
                    # kick + completion notification outweigh any
                    # descriptor-fetch pipelining
                    nc.sync.dma_start(
                        out=out[dc * P:(dc + 1) * P,
                                t * NT + s * w:t * NT + (s + 1) * w],
                        in_=y_s[:, :w],
                    )
    nc.compile()
    return nc


def _build_nc_f32r():
    """float32r variant: fp32 operands, ~tf32 matmul precision, weights
    streamed from HBM every token tile (both stacks can't stay resident
    in fp32).  DMA ~190 MB vs PE ~600us -> at the compute/memory ridge.
    Host pre-packs w1/w2/x into stream-block layouts so every streaming
    DMA is a fully linear copy."""
    from contextlib import ExitStack

    import concourse.mybir as mybir
    import concourse.tile as tile
    from concourse import bacc

    f32 = mybir.dt.float32
    f32r = mybir.dt.float32r
    AF = mybir.ActivationFunctionType

    FCB = FB // P  # fc groups per w1 block

    nc = bacc.Bacc(trn_type="TRN2")
    # pre-packed: xT[t,p,c,n], w1[fb,p,c,f], w2[dc,p,fc,d]
    xT = nc.dram_tensor("xT", [TT, P, DC, NT], f32r, kind="ExternalInput").ap()
    w1 = nc.dram_tensor("w1", [NFB, P, DC, FB], f32r, kind="ExternalInput").ap()
    w2 = nc.dram_tensor("w2", [DC, P, FC, P], f32r, kind="ExternalInput").ap()
    b1 = nc.dram_tensor("b1", [F_FF], f32, kind="ExternalInput").ap()
    b2 = nc.dram_tensor("b2", [D], f32, kind="ExternalInput").ap()
    out = nc.dram_tensor("out", [D, CAP], f32, kind="ExternalOutput").ap()

    with tile.TileContext(nc) as tc, ExitStack() as ctx:
        cpool = ctx.enter_context(tc.tile_pool(name="consts", bufs=1))
        xpool = ctx.enter_context(tc.tile_pool(name="xin", bufs=2))
        w1pool = ctx.enter_context(tc.tile_pool(name="w1s", bufs=3))
        w2pool = ctx.enter_context(tc.tile_pool(name="w2s", bufs=3))
        # h head (first 4 fc groups) is double-buffered so the next tile's
        # mm1 pipeline can restart while this tile's mm2 still reads h;
        # the 56 KB tail stays single-buffered (SBUF budget)
        HH = 4
        hhpool = ctx.enter_context(tc.tile_pool(name="hhead", bufs=2))
        hpool = ctx.enter_context(tc.tile_pool(name="hmid", bufs=1))
        ypool = ctx.enter_context(tc.tile_pool(name="yout", bufs=3))
        ppool = ctx.enter_context(tc.tile_pool(name="psum", bufs=4, space="PSUM"))

        warm = cpool.tile([P, NT], mybir.dt.bfloat16)
        nc.vector.memset(warm, 0.0)
        pwarm = ppool.tile([P, NT], f32, tag="ph")
        for _ in range(36):
            nc.tensor.matmul(pwarm, lhsT=warm[:, :P],
                             rhs=warm, start=True, stop=True)

        def load_x(t, split=1):
            xs = xpool.tile([P, DC, NT], f32r, name=f"x_s{t}", tag="xs")
            h = DC // split
            for s in range(split):
                nc.sync.dma_start(
                    out=xs[:, s * h:(s + 1) * h, :],
                    in_=xT[t, :, s * h:(s + 1) * h, :],
                )
            return xs

        def load_w1(fb, split=1):
            wb = w1pool.tile([P, DC, FB], f32r, name=f"w1b{fb}", tag="w1b")
            h = DC // split
            for s in range(split):
                nc.sync.dma_start(
                    out=wb[:, s * h:(s + 1) * h, :],
                    in_=w1[fb, :, s * h:(s + 1) * h, :],
                )
            return wb

        def load_w2(dc):
            wb = w2pool.tile([P, FC, P], f32r, name=f"w2b{dc}", tag="w2b")
            nc.sync.dma_start(out=wb, in_=w2[dc])
            return wb

        # critical startup prefix: x(0) and the first two w1 blocks
        x_cur = load_x(0, split=4)
        w1_q = [load_w1(0, split=4), load_w1(1, split=2)]

        b1_s = cpool.tile([P, FC], f32)
        nc.gpsimd.dma_start(out=b1_s, in_=b1.rearrange("(c p) -> p c", p=P))
        b2_s = cpool.tile([P, DC], f32)
        nc.gpsimd.dma_start(out=b2_s, in_=b2.rearrange("(c p) -> p c", p=P))

        def next_w1(t, fb):
            """Block to prefetch while (t, fb) is being consumed, keeping
            two blocks in flight."""
            nfb = fb + 2
            nt = t
            if nfb >= NFB:
                nfb -= NFB
                nt += 1
            return None if nt >= TT else nfb

        for t in range(TT):
            x_s = x_cur
            hh_s = hhpool.tile([P, HH, NT], f32r, name="hh_s", tag="hh")
            h_s = hpool.tile([P, FC - HH, NT], f32r)

            def h_at(fc):
                return hh_s[:, fc, :] if fc < HH else h_s[:, fc - HH, :]

            w2_q = []
            for fb in range(NFB):
                wb = w1_q.pop(0)
                pf = next_w1(t, fb)
                if pf is not None:
                    w1_q.append(load_w1(pf))
                if fb == 2 and t + 1 < TT:
                    # defer the next tile's x prefetch past the early w1
                    # blocks this tile's matmuls are waiting on
                    x_cur = load_x(t + 1)
                if fb == NFB - 2:
                    # issue mm2's first two w2 blocks late in mm1 so their
                    # transfers overlap the mm1 tail instead of its start
                    w2_q = [load_w2(0), load_w2(1)]
                for fcl in range(FCB):
                    fc = fb * FCB + fcl
                    ph = ppool.tile([P, NT], f32, name="ph", tag="ph")
                    for c in range(DC):
                        nc.tensor.matmul(
                            ph,
                            lhsT=wb[:, c, fcl * P:(fcl + 1) * P],
                            rhs=x_s[:, c, :],
                            start=(c == 0),
                            stop=(c == DC - 1),
                        )
                    nc.scalar.activation(
                        h_at(fc), ph, AF.Gelu, bias=b1_s[:, fc:fc + 1]
                    )
            for dc in range(DC):
                w2b = w2_q.pop(0)
                if dc + 2 < DC:
                    w2_q.append(load_w2(dc + 2))
                halves = 2 if (t == TT - 1 and dc == DC - 1) else 1
                w = NT // halves
                for s in range(halves):
                    py = ppool.tile([P, NT], f32, name="py", tag="py")
                    for fc in range(FC):
                        nc.tensor.matmul(
                            py[:, :w],
                            lhsT=w2b[:, fc, :],
                            rhs=h_at(fc)[:, s * w:(s + 1) * w],
                            start=(fc == 0),
                            stop=(fc == FC - 1),
                        )
                    y_s = ypool.tile([P, NT], f32, name="y_s", tag="ys")
                    nc.scalar.activation(
                        y_s[:, :w], py[:, :w], AF.Identity,
                        bias=b2_s[:, dc:dc + 1]
                    )
                    nc.sync.dma_start(
                        out=out[dc * P:(dc + 1) * P,
                                t * NT + s * w:t * NT + (s + 1) * w],
                        in_=y_s[:, :w],
                    )
    nc.compile()
    return nc


def _build_nc_strassen():
    """Per-core graph: 1-level Strassen <2,2,2> on both FFN matmuls,
    software-pipelined so mm2 of tile t interleaves with mm1 of tile t+1.

    Per 512-token tile (tok-halves a of 256): mm1 splits (tok, D, F) in
    half; 16 f-chunks x 7 products (4 k-tile MMs each, N=256) land in 4
    PSUM banks (2 products per bank, strictly sequential so the
    bank-granular pending-zero of start= is safe), the DVE combines them
    (one-PSUM-operand ops), ACT applies bias+gelu -> h.  mm2 mirrors it
    over (tok, F, D); its moving V forms are built JIT on the DVE from h
    in k-half units (plain blocks H11/H22 alias h directly, and the
    plain-form products run first so the DVE stays ahead).

    The interleave exists for DMA bandwidth: w2 forms stream 14.3MB/tile
    and back-to-back mm2 would need ~290GB/s; spread over a whole
    mm1+mm2 block it is ~255GB/s total alongside w1 forms 5-7 + x forms
    (w1 forms 1-4 stay resident).  h is double-buffered for the overlap;
    PSUM splits 4 mm1 banks + 4 mm2 banks, single-buffered each -- the
    interleave itself provides the drain slack.  Streams ride separate
    DGE rings (sync: mm1-critical, scalar: w2 forms + outputs) so a
    pool-gated descriptor only ever blocks its own stream's FIFO."""
    from contextlib import ExitStack

    import concourse.mybir as mybir
    import concourse.tile as tile
    from concourse import bacc

    bf = mybir.dt.bfloat16
    f32 = mybir.dt.float32
    AF = mybir.ActivationFunctionType
    ADD = mybir.AluOpType.add
    SUB = mybir.AluOpType.subtract

    nc = bacc.Bacc(trn_type="TRN2")
    xF = nc.dram_tensor("xF", [TT, 7, P, K1, HT], bf, kind="ExternalInput").ap()
    w1r = nc.dram_tensor("w1r", [JC, P, 4, K1, P], bf, kind="ExternalInput").ap()
    w1s = nc.dram_tensor("w1s", [JC, P, 3, K1, P], bf, kind="ExternalInput").ap()
    w2s = nc.dram_tensor("w2s", [DJ, 7, P, K2, P], bf, kind="ExternalInput").ap()
    b1 = nc.dram_tensor("b1", [P, FC], f32, kind="ExternalInput").ap()
    b2 = nc.dram_tensor("b2", [P, DC], f32, kind="ExternalInput").ap()
    out = nc.dram_tensor("out", [TT, DJ, 2, 2, P, HT], bf,
                         kind="ExternalOutput").ap()

    with tile.TileContext(nc) as tc, ExitStack() as ctx:
        wpool = ctx.enter_context(tc.tile_pool(name="wres", bufs=1))
        w1up = ctx.enter_context(tc.tile_pool(name="w1u", bufs=3))
        w2up = ctx.enter_context(tc.tile_pool(name="w2u", bufs=4))
        xfp = ctx.enter_context(tc.tile_pool(name="xf", bufs=2))
        hp = ctx.enter_context(tc.tile_pool(name="hh", bufs=2))
        vp = ctx.enter_context(tc.tile_pool(name="vv", bufs=3))
        tp = ctx.enter_context(tc.tile_pool(name="tmp", bufs=1))
        cp = ctx.enter_context(tc.tile_pool(name="cc", bufs=1))
        yp = ctx.enter_context(tc.tile_pool(name="yy", bufs=2))
        pp = ctx.enter_context(tc.tile_pool(name="ps", bufs=1, space="PSUM"))

        # PE warm-up: memset on GPSIMD (earliest preamble exit); junk burst
        # opens the HAM busy window until real data lands
        warm = wpool.tile([P, HT], bf)
        nc.gpsimd.memset(warm, 0.0)
        pj = pp.tile([P, 2 * HT], f32, name="pjunk", tag="bk0")
        for _ in range(24):
            nc.tensor.matmul(pj[:, :HT], lhsT=warm[:, :P], rhs=warm,
                             start=True, stop=True)

        w1r_s = wpool.tile([P, JC, 4, K1, P], bf)
        b1_s = wpool.tile([P, FC], f32)
        b2_s = wpool.tile([P, DC], f32)

        def load_w1r(j):
            nc.sync.dma_start(out=w1r_s[:, j], in_=w1r[j])

        def load_w1u(j):
            t = w1up.tile([P, 3, K1, P], bf, name="w1u", tag="w1u")
            nc.sync.dma_start(out=t, in_=w1s[j])
            return t

        def load_w2u(j, s):
            t = w2up.tile([P, K2, P], bf, name="w2u", tag="w2u")
            nc.scalar.dma_start(out=t, in_=w2s[j, s])
            return t

        def load_xf(t):
            xt = xfp.tile([P, 7, K1, HT], bf, name="xf", tag="xf")
            for i in range(7):
                nc.sync.dma_start(out=xt[:, i], in_=xF[t, i])
            return xt

        def combine(M):
            """PSUM->SBUF copies + one-PSUM-operand combine chains.
            M[f] = psum AP of product f.  Returns C11, C21, C12, C22."""
            tl = {n: tp.tile([P, HT], f32, name=n, tag=n)
                  for n in ("t1", "t2", "ta", "tc", "td")}
            co = {n: cp.tile([P, HT], f32, name=n, tag=n)
                  for n in ("c11", "c21", "c12", "c22")}
            tt = nc.vector.tensor_tensor
            nc.vector.tensor_copy(out=tl["t1"], in_=M[0])
            nc.vector.tensor_copy(out=tl["t2"], in_=M[1])
            tt(out=tl["ta"], in0=tl["t1"], in1=M[3], op=ADD)      # M1+M4
            tt(out=tl["ta"], in0=tl["ta"], in1=M[4], op=SUB)      # -M5
            tt(out=co["c11"], in0=tl["ta"], in1=M[6], op=ADD)     # +M7
            tt(out=co["c21"], in0=tl["t2"], in1=M[3], op=ADD)     # M2+M4
            tt(out=tl["tc"], in0=tl["t1"], in1=tl["t2"], op=SUB)  # M1-M2
            tt(out=tl["td"], in0=tl["tc"], in1=M[2], op=ADD)      # +M3
            tt(out=co["c22"], in0=tl["td"], in1=M[5], op=ADD)     # +M6
            tt(out=tl["td"], in0=tl["td"], in1=tl["tc"], op=SUB)  # = M3
            tt(out=co["c12"], in0=tl["td"], in1=M[4], op=ADD)     # M3+M5
            return co["c11"], co["c21"], co["c12"], co["c22"]

        class Mm1:
            """Emits mm1 of tile t one f-chunk at a time."""

            def __init__(self, t, xt, w1u):
                self.t, self.xt = t, xt
                self.h = hp.tile([P, FC, NT], bf, name="hh", tag="hh")
                self.w1u = w1u

            def chunk(self, j):
                nj = j + 3
                if nj < JC:
                    self.w1u[nj] = load_w1u(nj)
                wu = self.w1u.pop(j)
                banks = [pp.tile([P, 2 * HT], f32, name=f"bk{b_}",
                                 tag=f"bk{b_}") for b_ in range(4)]
                M = {}
                for s in range(7):
                    dst = banks[s // 2][:, (s % 2) * HT:(s % 2 + 1) * HT]
                    M[s] = dst
                    lt = (w1r_s[:, j, s] if s < 4 else wu[:, s - 4])
                    for k in range(K1):
                        nc.tensor.matmul(dst, lhsT=lt[:, k, :],
                                         rhs=self.xt[:, s, k, :],
                                         start=(k == 0), stop=(k == K1 - 1))
                c11, c21, c12, c22 = combine(M)
                for cc, fc, a_ in ((c11, j, 0), (c21, j, 1),
                                   (c12, 16 + j, 0), (c22, 16 + j, 1)):
                    nc.scalar.activation(
                        self.h[:, fc, a_ * HT:(a_ + 1) * HT], cc, AF.Gelu,
                        bias=b1_s[:, fc:fc + 1])

        class Mm2:
            """Emits mm2 of tile t one product at a time (28 products)."""

            def __init__(self, t, h, w2u):
                self.t, self.h, self.w2u = t, h, w2u
                self.banks = None
                self.M = {}

            def product(self, p):
                j, s = divmod(p, 7)
                f = SEQ2[s]
                h = self.h
                if s == 0:
                    self.banks = [pp.tile([P, 2 * HT], f32, name=f"mk{b_}",
                                          tag=f"mk{b_}") for b_ in range(4)]
                np_ = p + 2
                if np_ < DJ * 7 and np_ not in self.w2u:
                    self.w2u[np_] = load_w2u(*divmod(np_, 7))
                wu2 = self.w2u.pop(p)
                if f == 2:            # plain H11
                    rhs = lambda k: h[:, k, 0:HT]
                elif f == 3:          # plain H22
                    rhs = lambda k: h[:, 16 + k, HT:NT]
                else:
                    ttv = nc.vector.tensor_tensor
                    vts = []
                    for half in range(2):
                        vt = vp.tile([P, K2 // 2, HT], bf, name="vt", tag="vv")
                        vts.append(vt)
                        for kk in range(K2 // 2):
                            k = half * (K2 // 2) + kk
                            if f == 0:    # H11+H22
                                ttv(out=vt[:, kk, :], in0=h[:, k, 0:HT],
                                    in1=h[:, 16 + k, HT:NT], op=ADD)
                            elif f == 1:  # H21+H22
                                ttv(out=vt[:, kk, :], in0=h[:, k, HT:NT],
                                    in1=h[:, 16 + k, HT:NT], op=ADD)
                            elif f == 4:  # H11+H12
                                ttv(out=vt[:, kk, :], in0=h[:, k, 0:HT],
                                    in1=h[:, 16 + k, 0:HT], op=ADD)
                            elif f == 5:  # H21-H11
                                ttv(out=vt[:, kk, :], in0=h[:, k, HT:NT],
                                    in1=h[:, k, 0:HT], op=SUB)
                            else:         # H12-H22
                                ttv(out=vt[:, kk, :], in0=h[:, 16 + k, 0:HT],
                                    in1=h[:, 16 + k, HT:NT], op=SUB)
                    rhs = lambda k, v=vts: v[k // 8][:, k % 8, :]
                dst = self.banks[s // 2][:, (s % 2) * HT:(s % 2 + 1) * HT]
                self.M[f] = dst
                for k in range(K2):
                    nc.tensor.matmul(dst, lhsT=wu2[:, k, :], rhs=rhs(k),
                                     start=(k == 0), stop=(k == K2 - 1))
                if s == 6:
                    c11, c21, c12, c22 = combine(self.M)
                    self.M = {}
                    for cc, h2, a_ in ((c11, 0, 0), (c21, 0, 1),
                                       (c12, 1, 0), (c22, 1, 1)):
                        dc = h2 * 4 + j
                        ys = yp.tile([P, HT], bf, name="ys", tag="ys")
                        nc.scalar.activation(ys, cc, AF.Identity,
                                             bias=b2_s[:, dc:dc + 1])
                        nc.scalar.dma_start(out=out[self.t, j, h2, a_], in_=ys)

        # ---- t=0 startup prefix on the sync ring, consumption order
        xt0 = xfp.tile([P, 7, K1, HT], bf, name="xf", tag="xf")
        nc.sync.dma_start(out=xt0[:, 0], in_=xF[0, 0])
        load_w1r(0)
        pre_w1u = {0: load_w1u(0)}
        nc.sync.dma_start(out=xt0[:, 1], in_=xF[0, 1])
        nc.sync.dma_start(out=xt0[:, 2], in_=xF[0, 2])
        nc.sync.dma_start(out=b1_s, in_=b1)
        nc.sync.dma_start(out=b2_s, in_=b2)
        for i in range(3, 7):
            nc.sync.dma_start(out=xt0[:, i], in_=xF[0, i])
        for j in range(1, 3):
            load_w1r(j)
            pre_w1u[j] = load_w1u(j)
        for j in range(3, JC):
            load_w1r(j)

        # ---- prologue: mm1(0) alone (DMA-ramp-paced); stage mm2(0) units
        # and the next tile's x forms / first w1u units behind it
        mm1 = Mm1(0, xt0, pre_w1u)
        w2u_next, xf_next, w1u_next = {}, None, {}
        for j in range(JC):
            mm1.chunk(j)
            if j >= 10 and j - 10 < 5:
                w2u_next[j - 10] = load_w2u(*divmod(j - 10, 7))
            if j == 12 and TT > 1:
                xf_next = load_xf(1)
            if j == 13 and TT > 1:
                w1u_next = {jj: load_w1u(jj) for jj in (0, 1, 2)}
        h_prev = mm1.h

        # ---- pipelined blocks: B(t) = mm2(t) interleaved with mm1(t+1)
        for t in range(TT):
            mm2 = Mm2(t, h_prev, w2u_next)
            w2u_next = {}
            last = t + 1 >= TT
            if not last:
                mm1 = Mm1(t + 1, xf_next, w1u_next)
                h_prev = mm1.h
            p = 0
            for c in range(JC if not last else 1):
                if not last:
                    mm1.chunk(c)
                    if c == 9 and t + 2 < TT:
                        xf_next = load_xf(t + 2)
                    if c == 13 and t + 2 < TT:
                        w1u_next = {jj: load_w1u(jj) for jj in (0, 1, 2)}
                want = (c + 1) * (DJ * 7) // JC if not last else DJ * 7
                while p < want:
                    mm2.product(p)
                    p += 1
            while p < DJ * 7:
                mm2.product(p)
                p += 1
            # stage the next block's first mm2 units behind this block
            if not last:
                for s in range(2):
                    w2u_next[s] = load_w2u(*divmod(s, 7))
    nc.compile()
    return nc


def _pack_strassen(buf, w1e, b1e, b2e, w2e):
    """Host-side packing for one expert: dispatch buffer -> x A-forms,
    w1/w2 -> B-form streams, in exact device DMA layouts (partition-major)."""
    xFa = np.empty((TT, 7, P, K1, HT), dtype=BF16)
    for t in range(TT):
        xt = buf[t * NT:(t + 1) * NT]                    # [512, 1024]
        A11, A12 = xt[:HT, :512], xt[:HT, 512:]
        A21, A22 = xt[HT:, :512], xt[HT:, 512:]
        G = (A11 + A22, A21 + A22, A11, A22, A11 + A12, A21 - A11, A12 - A22)
        for i, g in enumerate(G):
            # g [256 tok, 512 D]: [p, k, n] = g.T[128k+p, n]
            xFa[t, i] = g.T.reshape(K1, P, HT).transpose(1, 0, 2)
    B11, B12 = w1e[:512, :2048], w1e[:512, 2048:]
    B21, B22 = w1e[512:, :2048], w1e[512:, 2048:]
    U = (B11 + B22, B11, B12 - B22, B21 - B11, B22, B11 + B12, B21 + B22)
    w1ra = np.empty((JC, P, 4, K1, P), dtype=BF16)
    w1sa = np.empty((JC, P, 3, K1, P), dtype=BF16)
    for j in range(JC):
        for i in range(7):
            # [k, p, m] -> [p, k, m]
            blk = U[i][:, j * P:(j + 1) * P].reshape(K1, P, P)
            if i < 4:
                w1ra[j, :, i] = blk.transpose(1, 0, 2)
            else:
                w1sa[j, :, i - 4] = blk.transpose(1, 0, 2)
    C11, C12 = w2e[:2048, :512], w2e[:2048, 512:]
    C21, C22 = w2e[2048:, :512], w2e[2048:, 512:]
    Z = (C11 + C22, C11, C12 - C22, C21 - C11, C22, C11 + C12, C21 + C22)
    w2sa = np.empty((DJ, 7, P, K2, P), dtype=BF16)
    for j in range(DJ):
        for s in range(7):
            w2sa[j, s] = Z[SEQ2[s]][:, j * P:(j + 1) * P] \
                .reshape(K2, P, P).transpose(1, 0, 2)
    return {
        "xF": np.ascontiguousarray(xFa), "w1r": np.ascontiguousarray(w1ra),
        "w1s": np.ascontiguousarray(w1sa), "w2s": np.ascontiguousarray(w2sa),
        "b1": np.ascontiguousarray(b1e.reshape(FC, P).T),
        "b2": np.ascontiguousarray(b2e.reshape(DC, P).T),
    }


def _build_nc_s1():
    """Hybrid: 1-level Strassen on mm1 + the baseline dense mm2.

    Combines only hardware-proven-clean pieces: the Strassen mm1 (N=256
    products, 7-in-4-banks co-packed PSUM, DVE combines, gelu -> the SAME
    h layout [P, FC, NT] the dense kernel uses) measured stall-free, and
    the dense mm2 (w2 resident, N=512 groups over h) is the baseline's.
    w2 streams nothing, so mm2-phase DMA is idle and stages the next
    tile's w1-form units and x forms.  w1 forms 1-3 resident, 4-7
    streamed per tile (8.2MB/tile ~ 168GB/s during mm1).  mm1 slots
    1120 + mm2 1280 = 2400 vs the dense kernel's 2560 (-6.3%)."""
    from contextlib import ExitStack

    import concourse.mybir as mybir
    import concourse.tile as tile
    from concourse import bacc

    bf = mybir.dt.bfloat16
    f32 = mybir.dt.float32
    AF = mybir.ActivationFunctionType
    ADD = mybir.AluOpType.add
    SUB = mybir.AluOpType.subtract

    NRES = 3   # resident w1 forms; 7-NRES streamed
    nc = bacc.Bacc(trn_type="TRN2")
    xF = nc.dram_tensor("xF", [TT, 7, P, K1, HT], bf, kind="ExternalInput").ap()
    w1r = nc.dram_tensor("w1r", [JC, P, NRES, K1, P], bf,
                         kind="ExternalInput").ap()
    w1s = nc.dram_tensor("w1s", [JC, P, 7 - NRES, K1, P], bf,
                         kind="ExternalInput").ap()
    # dense w2, dc-major so mm2 group dc needs only its 1MB slice
    w2 = nc.dram_tensor("w2", [DC, P, FC, P], bf, kind="ExternalInput").ap()
    b1 = nc.dram_tensor("b1", [P, FC], f32, kind="ExternalInput").ap()
    b2 = nc.dram_tensor("b2", [P, DC], f32, kind="ExternalInput").ap()
    out = nc.dram_tensor("out", [D, CAP], bf, kind="ExternalOutput").ap()

    with tile.TileContext(nc) as tc, ExitStack() as ctx:
        wpool = ctx.enter_context(tc.tile_pool(name="wres", bufs=1))
        w1up = ctx.enter_context(tc.tile_pool(name="w1u", bufs=4))
        xfp = ctx.enter_context(tc.tile_pool(name="xf", bufs=2))
        hp = ctx.enter_context(tc.tile_pool(name="hh", bufs=1))
        tp = ctx.enter_context(tc.tile_pool(name="tmp", bufs=1))
        cp = ctx.enter_context(tc.tile_pool(name="cc", bufs=1))
        yp = ctx.enter_context(tc.tile_pool(name="yy", bufs=4))
        pp = ctx.enter_context(tc.tile_pool(name="ps", bufs=2, space="PSUM"))

        warm = wpool.tile([P, HT], bf)
        nc.gpsimd.memset(warm, 0.0)
        pj = pp.tile([P, 2 * HT], f32, name="pjunk", tag="bk0")
        for _ in range(24):
            nc.tensor.matmul(pj[:, :HT], lhsT=warm[:, :P], rhs=warm,
                             start=True, stop=True)

        w1r_s = wpool.tile([P, JC, NRES, K1, P], bf)
        w2_s = wpool.tile([P, DC, FC, P], bf)
        b1_s = wpool.tile([P, FC], f32)
        b2_s = wpool.tile([P, DC], f32)

        def load_w1u(j):
            t = w1up.tile([P, 7 - NRES, K1, P], bf, name="w1u", tag="w1u")
            nc.sync.dma_start(out=t, in_=w1s[j])
            return t

        def load_xf(t):
            xt = xfp.tile([P, 7, K1, HT], bf, name="xf", tag="xf")
            for i in range(7):
                nc.sync.dma_start(out=xt[:, i], in_=xF[t, i])
            return xt

        # ---- t=0 prefix: per-chunk consumption order on the sync ring
        xt0 = xfp.tile([P, 7, K1, HT], bf, name="xf", tag="xf")
        w1u_t = {}
        nc.sync.dma_start(out=xt0[:, 0], in_=xF[0, 0])
        nc.sync.dma_start(out=w1r_s[:, 0], in_=w1r[0])
        w1u_t[0] = load_w1u(0)
        nc.sync.dma_start(out=xt0[:, 1], in_=xF[0, 1])
        nc.sync.dma_start(out=xt0[:, 2], in_=xF[0, 2])
        nc.sync.dma_start(out=b1_s, in_=b1)
        nc.sync.dma_start(out=b2_s, in_=b2)
        for i in range(3, 7):
            nc.sync.dma_start(out=xt0[:, i], in_=xF[0, i])
        for j in range(1, JC):
            nc.sync.dma_start(out=w1r_s[:, j], in_=w1r[j])
            if j < 4:
                w1u_t[j] = load_w1u(j)
            if j >= 8:
                # first two w2 slices ride the late prefix so mm2(0) dc 0-1
                # can start right after mm1(0)
                if j - 8 < 2:
                    nc.sync.dma_start(out=w2_s[:, j - 8], in_=w2[j - 8])

        def combine(M):
            tl = {n: tp.tile([P, HT], f32, name=n, tag=n)
                  for n in ("t1", "t2", "ta", "tc", "td")}
            co = {n: cp.tile([P, HT], f32, name=n, tag=n)
                  for n in ("c11", "c21", "c12", "c22")}
            tt = nc.vector.tensor_tensor
            nc.vector.tensor_copy(out=tl["t1"], in_=M[0])
            nc.vector.tensor_copy(out=tl["t2"], in_=M[1])
            tt(out=tl["ta"], in0=tl["t1"], in1=M[3], op=ADD)
            tt(out=tl["ta"], in0=tl["ta"], in1=M[4], op=SUB)
            tt(out=co["c11"], in0=tl["ta"], in1=M[6], op=ADD)
            tt(out=co["c21"], in0=tl["t2"], in1=M[3], op=ADD)
            tt(out=tl["tc"], in0=tl["t1"], in1=tl["t2"], op=SUB)
            tt(out=tl["td"], in0=tl["tc"], in1=M[2], op=ADD)
            tt(out=co["c22"], in0=tl["td"], in1=M[5], op=ADD)
            tt(out=tl["td"], in0=tl["td"], in1=tl["tc"], op=SUB)
            tt(out=co["c12"], in0=tl["td"], in1=M[4], op=ADD)
            return co["c11"], co["c21"], co["c12"], co["c22"]

        xf_next = xt0
        for t in range(TT):
            xt = xf_next
            h_s = hp.tile([P, FC, NT], bf)

            # ---- mm1: Strassen, 16 f-chunks x 7 products
            for j in range(JC):
                nj = j + 3
                if nj < JC and nj not in w1u_t:
                    w1u_t[nj] = load_w1u(nj)
                wu = w1u_t.pop(j)
                banks = [pp.tile([P, 2 * HT], f32, name=f"bk{b_}",
                                 tag=f"bk{b_}") for b_ in range(4)]
                M = {}
                for s in range(7):
                    dst = banks[s // 2][:, (s % 2) * HT:(s % 2 + 1) * HT]
                    M[s] = dst
                    lt = (w1r_s[:, j, s] if s < NRES else wu[:, s - NRES])
                    for k in range(K1):
                        nc.tensor.matmul(dst, lhsT=lt[:, k, :],
                                         rhs=xt[:, s, k, :],
                                         start=(k == 0), stop=(k == K1 - 1))
                c11, c21, c12, c22 = combine(M)
                for cc, fc, a_ in ((c11, j, 0), (c21, j, 1),
                                   (c12, 16 + j, 0), (c22, 16 + j, 1)):
                    nc.scalar.activation(
                        h_s[:, fc, a_ * HT:(a_ + 1) * HT], cc, AF.Gelu,
                        bias=b1_s[:, fc:fc + 1])

            # ---- mm2: dense (the baseline's), w2 resident, N=512 groups
            for dc in range(DC):
                if t == 0 and dc + 2 < DC:
                    nc.sync.dma_start(out=w2_s[:, dc + 2], in_=w2[dc + 2])
                if dc == 2 and t + 1 < TT:
                    xf_next = load_xf(t + 1)
                if dc == 4 and t + 1 < TT:
                    w1u_t[0] = load_w1u(0)
                    w1u_t[1] = load_w1u(1)
                if dc == 6 and t + 1 < TT:
                    w1u_t[2] = load_w1u(2)
                halves = 2 if (t == TT - 1 and dc == DC - 1) else 1
                w = NT // halves
                for s_ in range(halves):
                    py = pp.tile([P, NT], f32, name="py", tag=f"bk{dc % 4}")
                    for fc in range(FC):
                        nc.tensor.matmul(
                            py[:, :w],
                            lhsT=w2_s[:, dc, fc, :],
                            rhs=h_s[:, fc, s_ * w:(s_ + 1) * w],
                            start=(fc == 0), stop=(fc == FC - 1),
                        )
                    y_s = yp.tile([P, NT], bf, name="y_s", tag="ys")
                    nc.scalar.activation(y_s[:, :w], py[:, :w], AF.Identity,
                                         bias=b2_s[:, dc:dc + 1])
                    nc.sync.dma_start(
                        out=out[dc * P:(dc + 1) * P,
                                t * NT + s_ * w:t * NT + (s_ + 1) * w],
                        in_=y_s[:, :w],
                    )
    nc.compile()
    return nc


def _pack_s1(buf, w1e, b1e, b2e, w2e):
    """Host packing for the s1 hybrid: Strassen x/w1 forms + dense w2."""
    NRES = 3
    xFa = np.empty((TT, 7, P, K1, HT), dtype=BF16)
    for t in range(TT):
        xt = buf[t * NT:(t + 1) * NT]
        A11, A12 = xt[:HT, :512], xt[:HT, 512:]
        A21, A22 = xt[HT:, :512], xt[HT:, 512:]
        G = (A11 + A22, A21 + A22, A11, A22, A11 + A12, A21 - A11, A12 - A22)
        for i, g in enumerate(G):
            xFa[t, i] = g.T.reshape(K1, P, HT).transpose(1, 0, 2)
    B11, B12 = w1e[:512, :2048], w1e[:512, 2048:]
    B21, B22 = w1e[512:, :2048], w1e[512:, 2048:]
    U = (B11 + B22, B11, B12 - B22, B21 - B11, B22, B11 + B12, B21 + B22)
    w1ra = np.empty((JC, P, NRES, K1, P), dtype=BF16)
    w1sa = np.empty((JC, P, 7 - NRES, K1, P), dtype=BF16)
    for j in range(JC):
        for i in range(7):
            blk = U[i][:, j * P:(j + 1) * P].reshape(K1, P, P)
            if i < NRES:
                w1ra[j, :, i] = blk.transpose(1, 0, 2)
            else:
                w1sa[j, :, i - NRES] = blk.transpose(1, 0, 2)
    w2p = np.ascontiguousarray(
        w2e.reshape(FC, P, DC, P).transpose(2, 1, 0, 3)).astype(BF16)
    return {
        "xF": np.ascontiguousarray(xFa), "w1r": np.ascontiguousarray(w1ra),
        "w1s": np.ascontiguousarray(w1sa), "w2": w2p,
        "b1": np.ascontiguousarray(b1e.reshape(FC, P).T),
        "b2": np.ascontiguousarray(b2e.reshape(DC, P).T),
    }


def _route(x, w_router, b_router):
    """Replicates reference routing (softmax -> top-2 -> capacity) in f64.

    Returns per-expert (token_ids, slot_positions, gate_values)."""
    xf = x.reshape(N, D).astype(np.float64)
    logits = xf @ w_router.astype(np.float64) + b_router.astype(np.float64)
    logits -= logits.max(axis=-1, keepdims=True)
    p = np.exp(logits)
    gates = p / p.sum(axis=-1, keepdims=True)
    # top-2, ties to the lower index (matches lax.top_k)
    order = np.argsort(-gates, axis=1, kind="stable")[:, :TOP_K]
    topv = np.take_along_axis(gates, order, axis=1)
    e_flat = order.reshape(-1)
    g_flat = topv.reshape(-1).astype(np.float32)
    tok = np.repeat(np.arange(N), TOP_K)
    pos = np.empty(N * TOP_K, np.int64)
    for e in range(E):
        m_e = e_flat == e
        pos[m_e] = np.arange(int(m_e.sum()))
    keep = pos < CAP
    per_expert = []
    for e in range(E):
        sel = (e_flat == e) & keep
        per_expert.append((tok[sel], pos[sel], g_flat[sel]))
    return per_expert


def _run_device(in_maps, trace=False):
    from concourse.bass_utils import run_bass_kernel_spmd

    if MODE not in _NC_CACHE:
        _NC_CACHE[MODE] = {"bf16": _build_nc, "f32r": _build_nc_f32r,
                           "strassen": _build_nc_strassen,
                           "s1": _build_nc_s1}[MODE]()
    return run_bass_kernel_spmd(
        _NC_CACHE[MODE], in_maps, core_ids=list(range(NCORES)), trace=trace
    )


def _kernel_impl(inputs, trace=False):
    x = np.asarray(inputs["x"], dtype=np.float32)
    w_router = np.asarray(inputs["w_router"], dtype=np.float32)
    b_router = np.asarray(inputs["b_router"], dtype=np.float32)
    w1 = np.asarray(inputs["w1"], dtype=np.float32)
    b1 = np.ascontiguousarray(np.asarray(inputs["b1"], dtype=np.float32))
    w2 = np.asarray(inputs["w2"], dtype=np.float32)
    b2 = np.ascontiguousarray(np.asarray(inputs["b2"], dtype=np.float32))

    per_expert = _route(x, w_router, b_router)
    xf = x.reshape(N, D)

    in_maps = []
    for e in range(E):
        tk, ps, _ = per_expert[e]
        buf = np.zeros((CAP, D), np.float32)
        buf[ps] = xf[tk]
        if MODE == "strassen":
            in_maps.append(_pack_strassen(buf, w1[e], b1[e], b2[e], w2[e]))
            continue
        if MODE == "s1":
            in_maps.append(_pack_s1(buf, w1[e], b1[e], b2[e], w2[e]))
            continue
        bufT = np.ascontiguousarray(buf.T)          # [D, CAP]
        if MODE == "bf16":
            # per-descriptor-linear layouts: x[t,c,p,n], w1[fb,cpair,p,2,f],
            # w2[cg,p,4,d]
            xp = np.ascontiguousarray(
                bufT.astype(BF16).reshape(DC, P, TT, NT).transpose(2, 0, 1, 3))
            w1p = np.ascontiguousarray(
                w1[e].astype(BF16).reshape(DC // 2, 2, P, NFB, FB)
                .transpose(3, 0, 2, 1, 4))
            w2p = np.ascontiguousarray(
                w2[e].astype(BF16).reshape(FC // 4, 4, P, D)
                .transpose(0, 2, 1, 3))
            in_maps.append({
                "xT": xp, "w1": w1p, "w2": w2p,
                "b1": np.ascontiguousarray(b1[e].reshape(FC, P).T),
                "b2": np.ascontiguousarray(b2[e].reshape(DC, P).T),
            })
        else:
            # stream-block layouts: x[t,p,c,n], w1[fb,p,c,f], w2[dc,p,fc,d]
            xp = np.ascontiguousarray(
                bufT.reshape(DC, P, TT, NT).transpose(2, 1, 0, 3))
            w1p = np.ascontiguousarray(
                w1[e].reshape(DC, P, NFB, FB).transpose(2, 1, 0, 3))
            w2p = np.ascontiguousarray(
                w2[e].reshape(FC, P, DC, P).transpose(2, 1, 0, 3))
            in_maps.append({
                "xT": xp, "w1": w1p, "w2": w2p, "b1": b1[e], "b2": b2[e],
            })

    res = _run_device(in_maps, trace=trace)

    y = np.zeros((N, D), np.float32)
    ws = np.zeros((N,), np.float32)
    for e in range(E):
        tk, ps, gv = per_expert[e]
        outT = np.asarray(res.results[e]["out"], dtype=np.float32)
        if MODE == "strassen":
            # [TT, DJ, 2, 2, P, HT] -> [D, CAP]: D = (h2*4+j)*128+p,
            # tok = 512t + 256a + n
            outT = outT.transpose(2, 1, 4, 0, 3, 5).reshape(D, CAP)
        vals = (outT[:, ps] * gv[None, :]).T  # [n_e, D]
        y[tk] += vals                         # tk unique within one expert
        ws[tk] += gv
    y = np.where((ws > 0.0)[:, None], y / np.maximum(ws, 1e-6)[:, None], y)
    return y.reshape(B, T, D).astype(np.float32), res


def kernel(**inputs):
    y, _ = _kernel_impl(inputs, trace=False)
    return y

